# revision 36
# baseline (speedup 1.0000x reference)
"""Transformer decoder layer (self-attn + cross-attn + FFN, post-LN) on 8
Trainium2 NeuronCores, sequence-parallel with zero collectives, with fp8
DoubleRow matmuls for the large GEMMs.

Sharding: core c -> batch b = c//4, causal-balanced chunk pair (j, 7-j) of
256 tokens each (j = c%4), so every core owns 512 query tokens with equal
total causal attention area. Weights are replicated; K/V projections are
recomputed per core. All per-core differences are expressed through input
DATA (token reordering + additive exp-bias masks), so a single SPMD program
serves all 8 cores.

Quantization (validated to rel-err ~1e-2 vs the 2e-2 gate):
- Q/K/V projections (self+cross): W single-fp8 x X hi/lo-fp8, DoubleRow
  k-packed (contract 256/instr at 0.5 cyc/row) -> 2x PE vs bf16.
- cross-Q: W hi/lo x x1n single-fp8 (2-term).
- scores: K single-fp8 (stride-0 slot broadcast) x Q exact (hi+lo slots),
  one DoubleRow instr per s-tile -> 2x.
- attn*V: exp output directly fp8, V fp8, DoubleRow s-tile pairs -> 4x.
- W_O self/cross: bf16 (attnT unquantized, saves error budget).
- FFN: 3-term hi/lo x hi/lo (drop lo*lo) -> 1.33x.
Scales (powers of 2, exact): X*8, W*64, V*4, h*8; all folded into
activation scales / copy scalars; K-bias dropped (softmax-invariant),
V-bias folded into the attnT write, b2 folded into LN2's beta with
b1' = b1 - b2 @ W1 compensating in the FFN.
"""

import sys

if "/opt/trn_rl_repo" not in sys.path:
    sys.path.insert(0, "/opt/trn_rl_repo")

from contextlib import ExitStack

import numpy as np
import ml_dtypes

import concourse.bass as bass
import concourse.bacc as bacc
import concourse.tile as tile
import concourse.mybir as mybir
from concourse.bass_utils import run_bass_kernel_spmd
from concourse.masks import make_identity

F32 = mybir.dt.float32
BF16 = mybir.dt.bfloat16
F8 = mybir.dt.float8e4
U8 = mybir.dt.uint8
AF = mybir.ActivationFunctionType
ALU = mybir.AluOpType
DR = mybir.MatmulPerfMode.DoubleRow

NP8 = ml_dtypes.float8_e4m3
NPBF = ml_dtypes.bfloat16

D = 1024
H = 16
DK = 64
DFF = 4096
B = 2
T = 2048
N_CORES = 8
CHUNK = 256
TQ = 512          # query tokens per core
KV = 2048         # padded kv layout length (self), enc length (cross)
FT = D // 128     # 8 f-tiles
HT = DFF // 128   # 32 ffn tiles
NSEG = 8          # kv/enc DMA-streaming segments of 256 tokens
NEG = -50.0       # additive pre-exp mask (exp(-50) ~ 2e-22)

SX = 8.0          # activation fp8 scale
SW = 64.0         # weight fp8 scale
SV = 4.0          # V fp8 scale
SH = 8.0          # ffn hidden fp8 scale
PSC = SX * SW     # psum scale of an X*W product (512)
EXPA = float(8.0 / np.log(2.0))   # fp8-Schraudolph exp slope
EXPB = 55.5                       # fp8-Schraudolph exp offset (e4m3 bias)

# self-attn 256-token s-block schedules over the kv layout
# [A(256) | B(256) | rest... | pad]  (block = 2 s-tiles of 128):
BLOCKS_A = [0, 2, 3, 4]    # own diag + 768-token prior window
BLOCKS_B = list(range(8))  # everything (pads masked via bias)

_BUILT = None
_NC = None


def _build():
    nc = bacc.Bacc("TRN2", target_bir_lowering=False, debug=False,
                   num_devices=N_CORES)

    def din(name, shape, dt):
        return nc.dram_tensor(name, shape, dt, kind="ExternalInput").ap()

    xq_d = din("xq", [128, FT, 2, TQ], F8)          # planes (lo, hi), *SX
    xres_d = din("xres", [128, FT, TQ], F32)
    xkv_d = din("xkv", [NSEG, 128, FT, 2, 256], F8)  # seg-major, *SX
    enc_d = din("enc", [NSEG, 128, FT, 2, 256], F8)
    w_d = {}
    for nm in ("wq_s", "wk_s", "wv_s", "wk_c", "wv_c"):
        w_d[nm] = din(nm, [128, FT, D], F8)          # *SW
    wqc_d = din("wq_c", [128, 2, FT, D], F8)         # planes (hi, lo), *SW
    wo_s_d = din("wo_s", [128, FT, D], BF16)
    wo_c_d = din("wo_c", [128, FT, D], BF16)
    w1_d = din("w1", [128, 2, FT, DFF], F8)          # planes (hi, lo), *SW
    w2_d = din("w2", [128, 2, HT, D], F8)            # planes (hi, lo), *SW
    # small fp32 consts packed into one tensor:
    # bo_s(8) bo_c(8) bv_s(8) bv_c(8) g1 be1 g2 be2'(incl b2) g3 be3 (48)
    # b1'(32) biasa(8) biasb(8)  -> 128 columns
    BIAS_NAMES = ("bo_s", "bo_c", "bv_s", "bv_c",
                  "g1", "be1", "g2", "be2", "g3", "be3")
    smallf_d = din("smallf", [128, 10 * FT + HT + 32], F32)
    qrow_d = din("qrow", [1, 2 * D], BF16)           # bq_s*PSC, bq_c*PSC
    dmb_d = din("dmb", [128, 2, 256], F32)           # 0 / -1e5 causal bias
    out_d = nc.dram_tensor("out", [128, FT, TQ], F32, kind="ExternalOutput").ap()

    with tile.TileContext(nc) as tc, ExitStack() as S:
        const = S.enter_context(tc.tile_pool(name="const", bufs=1))
        pp = S.enter_context(tc.tile_pool(name="ps", bufs=1, space="PSUM"))
        resid = S.enter_context(tc.tile_pool(name="resid", bufs=1))

        ident = const.tile([128, 128], BF16)
        make_identity(nc, ident)
        ones_b = const.tile([128, 1], BF16)
        nc.vector.memset(ones_b, 1.0)
        ones_row = const.tile([1, 128], F32)
        nc.vector.memset(ones_row, 1.0)
        ones_q = const.tile([1, TQ], BF16)
        nc.vector.memset(ones_q, 1.0)
        eps_t = const.tile([1, 1], F32)
        nc.vector.memset(eps_t, 1e-5)

        glob_ctx = ExitStack()
        glob = glob_ctx.enter_context(tc.tile_pool(name="glob", bufs=1))

        # =========== helpers ===========
        PS_BUFS = {"big": 2, "st": 3, "av": 2, "t": 1}

        def ps_tile(tag, bufs, shape=(128, 512), dt=F32, name="ps"):
            return pp.tile(list(shape), dt, tag=tag, bufs=PS_BUFS[tag],
                           name=name)

        def wtile8(nm, dram):
            t = glob.tile([128, FT, D], F8, tag="wstream", bufs=2, name=nm)
            for dc in range(FT):     # per-chunk so first consumers start early
                nc.sync.dma_start(out=t[:, dc, :], in_=dram[:, dc, :])
            return t

        TAG8 = ["big", "big", "st", "st", "st", "av", "av", "t"]

        def proj_q8(QT, W_sb, X_sb, qoff, lbl, whl=False):
            """Q projection -> QT [128, FT, 2, TQ] fp8 planes (lo, hi).
            whl=False: W single fp8, X_sb planes (lo,hi) [128,FT,2,TQ].
            whl=True:  W planes (hi,lo) [128,2,FT,D], X single [128,FT,TQ].
            Adds bq via a bf16 bias-row matmul."""
            ps8 = [ps_tile(TAG8[ft], 0, name=f"pjq_{lbl}_{ft}")
                   for ft in range(FT)]
            for ft in range(FT):
                nc.tensor.matmul(
                    ps8[ft], lhsT=qrow_sb[0:1, qoff + ft * 128:
                                          qoff + (ft + 1) * 128],
                    rhs=ones_q, start=True, stop=False)
            for p in range(2):
                for dcp in range(4):
                    if whl:   # stream the W plane/dc-pair piece
                        wqp = glob.tile([128, 2, D], F8, tag="wqc", bufs=2,
                                        name=f"wqp_{p}_{dcp}")
                        nc.sync.dma_start(
                            out=wqp, in_=W_sb[:, p, 2 * dcp:2 * dcp + 2, :])
                    for ft in range(FT):
                        if whl:
                            lhsT = wqp[:, :, ft * 128:(ft + 1) * 128]
                            rhs = X_sb[:, 2 * dcp:2 * dcp + 2, :]
                        else:
                            lhsT = W_sb[:, 2 * dcp:2 * dcp + 2,
                                        ft * 128:(ft + 1) * 128]
                            rhs = X_sb[:, 2 * dcp:2 * dcp + 2, p, :]
                        nc.tensor.matmul(
                            ps8[ft], lhsT=lhsT, rhs=rhs, start=False,
                            stop=(p == 1 and dcp == 3), perf_mode=DR)
            for ft in range(FT):
                # hi plane on Act (psum scale PSC -> *SX), lo residual on DVE
                nc.scalar.activation(out=QT[:, ft, 1, :], in_=ps8[ft],
                                     func=AF.Copy, scale=SX / PSC)
                nc.vector.scalar_tensor_tensor(
                    out=QT[:, ft, 0, :], in0=ps8[ft], scalar=SX / PSC,
                    in1=QT[:, ft, 1, :], op0=ALU.mult, op1=ALU.subtract)

        def _kv_units(KT, V_list, seg, X_piece, WK_sb, WV_sb, vtag,
                      planes=(0, 1)):
            """4 sub-units (2 V-halves, 2 K-halves) for one 256-token seg.
            X_piece [128, FT, 2, 256] planes (lo, hi); planes=(1,) drops the
            lo residual (single-quant X, ~2x fewer matmuls)."""
            sl = slice(seg * 256, (seg + 1) * 256)
            vt = glob.tile([128, 2, H, DK + 1], F8, tag="v", bufs=16,
                           name=f"v_{vtag}_{seg}")
            V_list.append(vt)

            def v_unit(sti):
                for half in range(2):
                    ps = ps_tile("big", 0,
                                 name=f"pv_{vtag}_{seg}_{sti}_{half}")
                    for pi, p in enumerate(planes):
                        for dcp in range(4):
                            nc.tensor.matmul(
                                ps,
                                lhsT=X_piece[:, 2 * dcp:2 * dcp + 2, p,
                                             sti * 128:(sti + 1) * 128],
                                rhs=WV_sb[:, 2 * dcp:2 * dcp + 2,
                                          half * 512:(half + 1) * 512],
                                start=(pi == 0 and dcp == 0),
                                stop=(pi == len(planes) - 1 and dcp == 3),
                                perf_mode=DR)
                    nc.vector.tensor_scalar_mul(
                        out=vt[:, sti, half * 8:(half + 1) * 8, 0:DK],
                        in0=ps.rearrange("p (a b) -> p a b", b=DK),
                        scalar1=SV / PSC)
                if sti == 1:
                    nc.vector.memset(vt[:, :, :, DK:DK + 1], 1.0)

            def k_unit(fpp):
                for ftp in (fpp * 2, fpp * 2 + 1):
                    ps = ps_tile("st", 0, shape=(128, 2, 256),
                                 name=f"pjk_{vtag}_{seg}_{ftp}")
                    for fi in range(2):
                        ft = ftp * 2 + fi
                        for pi, p in enumerate(planes):
                            for dcp in range(4):
                                nc.tensor.matmul(
                                    ps[:, fi, :],
                                    lhsT=WK_sb[:, 2 * dcp:2 * dcp + 2,
                                               ft * 128:(ft + 1) * 128],
                                    rhs=X_piece[:, 2 * dcp:2 * dcp + 2, p, :],
                                    start=(pi == 0 and dcp == 0),
                                    stop=(pi == len(planes) - 1 and dcp == 3),
                                    perf_mode=DR)
                    nc.vector.tensor_scalar_mul(
                        out=KT[:, ftp * 2:ftp * 2 + 2, sl], in0=ps,
                        scalar1=SX / PSC)

            return [lambda: v_unit(0), lambda: v_unit(1),
                    lambda: k_unit(0), lambda: k_unit(1)]

        def proj_kv_seg8(KT, V_list, seg, X_piece, WK_sb, WV_sb, vtag,
                         planes=(0, 1)):
            for u in _kv_units(KT, V_list, seg, X_piece, WK_sb, WV_sb, vtag,
                               planes):
                u()

        # Normalized attention tiles go through a PE transpose whose input
        # comes from a short DVE chain; the transposes are deferred into the
        # NEXT head's PE stream to avoid stalling the in-order PE queue.
        pending_t = []
        _tcnt = [0]

        def _norm1(psav, attnT, po, fp, q0, bv_t, nm):
            rec = glob.tile([128, 1], F32, tag="rec", bufs=8, name=f"r{nm}")
            nc.vector.reciprocal(rec, psav[:, DK:DK + 1])
            an = glob.tile([128, DK], BF16, tag="an", bufs=8, name=f"n{nm}")
            nc.vector.tensor_scalar_mul(an, psav[:, 0:DK], rec)
            pending_t.append((an, attnT, po, fp, q0, bv_t))

        def flush_t():
            """Group the 4 pending (2 heads x 2 qt) tiles per (fp, 256-q
            span) into one PSUM tile and a single biased Act copy."""
            groups = {}
            for an, attnT, po, fp, q0, bv_t in pending_t:
                key = (id(attnT), fp, q0 - q0 % 256)
                groups.setdefault(key, []).append((an, attnT, po, fp, q0,
                                                   bv_t))
            for key, items in groups.items():
                _tcnt[0] += 1
                if len(items) == 4:
                    pst = ps_tile("t", 0, shape=(128, 256), dt=BF16,
                                  name=f"pt{_tcnt[0]}")
                    _, attnT, _, fp, _, bv_t = items[0]
                    qb = key[2]
                    for an, _, po, _, q0, _ in items:
                        nc.tensor.transpose(
                            pst[po:po + DK, q0 - qb:q0 - qb + 128], an, ident)
                    nc.scalar.activation(
                        out=attnT[:, fp, qb:qb + 256], in_=pst,
                        func=AF.Identity, scale=1.0 / SV,
                        bias=bv_t[:, fp:fp + 1])
                else:
                    for an, attnT, po, fp, q0, bv_t in items:
                        pst = ps_tile("t", 0, shape=(DK, 128), dt=BF16,
                                      name=f"pt{_tcnt[0]}")
                        nc.tensor.transpose(pst, an, ident)
                        nc.scalar.activation(
                            out=attnT[po:po + DK, fp, q0:q0 + 128], in_=pst,
                            func=AF.Identity, scale=1.0 / SV,
                            bias=bv_t[po:po + DK, fp:fp + 1])
            pending_t.clear()

        def sc_matmul(ps_out, KT, QT, po, fp, st, qsl, start=True):
            kb = KT[po:po + DK, fp, st * 128:(st + 1) * 128] \
                .unsqueeze(1).broadcast_to((DK, 2, 128))
            nc.tensor.matmul(ps_out, lhsT=kb,
                             rhs=QT[po:po + DK, fp, :, qsl],
                             start=start, stop=True, perf_mode=DR,
                             skip_group_check=not start)

        def attn_chunk(QT, KT, V_list, attnT, bv_t, cn, qoff, blocks, bias2,
                       bias2dve, dveset, diag_blk, filler=None):
            qsl = slice(qoff, qoff + CHUNK)
            for h in range(H):
                fp, po = h // 2, (h % 2) * DK
                ats = {}
                for blk in blocks:
                    ps = ps_tile("st", 0, name=f"pss_{h}_{cn}_{blk}")
                    for half in range(2):
                        sc_matmul(ps[:, half * 256:(half + 1) * 256],
                                  KT, QT, po, fp, blk * 2 + half, qsl)
                    if blk == diag_blk:
                        psv = ps.rearrange("p (a b) -> p a b", b=256)
                        nc.vector.tensor_tensor(out=psv, in0=psv, in1=dmb_sb,
                                                op=ALU.add)
                    dve_exp = blk in dveset
                    at = glob.tile([128, 2, 256], U8 if dve_exp else F8,
                                   tag="at", bufs=10, name=f"a_{h}_{cn}_{blk}")
                    if dve_exp:
                        nc.vector.tensor_scalar(
                            out=at.rearrange("p a b -> p (a b)"), in0=ps,
                            scalar1=EXPA * 0.125 / (SX * SX),
                            scalar2=bias2dve[:, blk:blk + 1],
                            op0=ALU.mult, op1=ALU.add)
                    else:
                        nc.scalar.activation(
                            out=at.rearrange("p a b -> p (a b)"), in_=ps,
                            func=AF.Exp, scale=0.125 / (SX * SX),
                            bias=bias2[:, blk:blk + 1])
                    ats[blk] = at.bitcast(F8) if dve_exp else at
                if h % 2 == 0:
                    flush_t()
                for qt in range(2):
                    psav = ps_tile("av", 0, shape=(128, DK + 1),
                                   name=f"pav_{h}_{cn}_{qt}")
                    for i, blk in enumerate(blocks):
                        nc.tensor.matmul(
                            psav,
                            lhsT=ats[blk][:, :, qt * 128:(qt + 1) * 128],
                            rhs=V_list[blk][:, :, h, :],
                            start=(i == 0), stop=(i == len(blocks) - 1),
                            perf_mode=DR)
                    _norm1(psav, attnT, po, fp, qoff + qt * 128, bv_t,
                           f"s_{h}_{cn}_{qt}")
                if filler is not None:
                    filler()
                    filler()
            flush_t()

        def attn_cross(QT, KT, V_list, attnT, bv_t):
            # two half-passes over s so only 8 exp tiles are live per head
            for h in range(H):
                fp, po = h // 2, (h % 2) * DK
                psavs = [ps_tile("av" if qt % 2 == 0 else "big", 0,
                                 shape=(128, DK + 1), name=f"pavc_{h}_{qt}")
                         for qt in range(4)]
                for half in range(2):
                    ats = {}
                    for pr in range(half * 4, half * 4 + 4):
                        # pairs 1/4/6 compute exp on DVE via the fp8
                        # Schraudolph bit trick (uint8(A*x+B) bitcast to
                        # e4m3); rest stay on the Act engine. Splits the
                        # exp load across both engines.
                        dve_exp = pr in (1, 4, 6)
                        at = glob.tile([128, 2, TQ], U8 if dve_exp else F8,
                                       tag="atc", bufs=5, name=f"ac_{h}_{pr}")
                        for sti in range(2):
                            st = pr * 2 + sti
                            ps = ps_tile("st", 0, name=f"psc_{h}_{st}")
                            sc_matmul(ps, KT, QT, po, fp, st,
                                      slice(0, TQ))
                            if dve_exp:
                                nc.vector.tensor_scalar(
                                    out=at[:, sti, :], in0=ps,
                                    scalar1=EXPA * 0.125 / (SX * SX),
                                    scalar2=EXPB,
                                    op0=ALU.mult, op1=ALU.add)
                            else:
                                nc.scalar.activation(
                                    out=at[:, sti, :], in_=ps, func=AF.Exp,
                                    scale=0.125 / (SX * SX))
                        ats[pr] = at.bitcast(F8) if dve_exp else at
                    if half == 0 and h % 2 == 0:
                        flush_t()
                    for qt in range(4):
                        for pr in range(half * 4, half * 4 + 4):
                            nc.tensor.matmul(
                                psavs[qt],
                                lhsT=ats[pr][:, :, qt * 128:(qt + 1) * 128],
                                rhs=V_list[pr][:, :, h, :],
                                start=(pr == 0), stop=(pr == 7),
                                perf_mode=DR)
                for qt in range(4):
                    _norm1(psavs[qt], attnT, po, fp, qt * 128, bv_t,
                           f"c_{h}_{qt}")
            flush_t()

        def wo_resid(attnT, wo_dram, bo_t, x_prev, x_out, lbl):
            for fp2 in range(4):
                wop = glob.tile([128, FT, 256], BF16, tag="wop", bufs=2,
                                name=f"wo_{lbl}_{fp2}")
                nc.sync.dma_start(
                    out=wop, in_=wo_dram[:, :, fp2 * 256:(fp2 + 1) * 256])
                for f2 in range(2):
                    fo = fp2 * 2 + f2
                    ps = ps_tile("big" if fo % 2 == 0 else "st", 0,
                                 name=f"pwo_{lbl}_{fo}")
                    for fi in range(FT):
                        nc.tensor.matmul(
                            ps, lhsT=wop[:, fi, f2 * 128:(f2 + 1) * 128],
                            rhs=attnT[:, fi, :],
                            start=(fi == 0), stop=(fi == FT - 1))
                    nc.vector.scalar_tensor_tensor(
                        out=x_out[:, fo, :], in0=ps,
                        scalar=bo_t[:, fo:fo + 1],
                        in1=x_prev[:, fo, :], op0=ALU.add, op1=ALU.add)

        def ln_stats(x_in, lbl):
            """-> (ps_mu, ps_rstd) broadcast PSUM tiles."""
            ps_sum = ps_tile("st", 0, shape=(1, TQ), name=f"psum_{lbl}")
            ps_sq = ps_tile("big", 0, shape=(1, TQ), name=f"psq_{lbl}")
            for fc in range(FT):
                xb = resid.tile([128, TQ], BF16, tag="sqb", bufs=3,
                                name=f"xb_{lbl}_{fc}")
                nc.gpsimd.tensor_copy(out=xb, in_=x_in[:, fc, :])
                nc.tensor.matmul(ps_sum, lhsT=ones_b, rhs=xb,
                                 start=(fc == 0), stop=(fc == FT - 1))
                sqb = resid.tile([128, TQ], BF16, tag="sqb", bufs=3,
                                 name=f"sq_{lbl}_{fc}")
                nc.vector.tensor_mul(sqb, xb, xb)
                nc.tensor.matmul(ps_sq, lhsT=ones_b, rhs=sqb,
                                 start=(fc == 0), stop=(fc == FT - 1))
            return ln_finish(ps_sum, ps_sq, lbl)

        def ln_finish(ps_sum, ps_sq, lbl):
            mu = resid.tile([1, TQ], F32, tag="stat", bufs=2, name=f"mu_{lbl}")
            nc.scalar.activation(out=mu, in_=ps_sum, func=AF.Copy, scale=1.0 / D)
            msq = resid.tile([1, TQ], F32, tag="stat", bufs=2,
                             name=f"msq_{lbl}")
            nc.scalar.activation(out=msq, in_=ps_sq, func=AF.Copy, scale=1.0 / D)
            mu2 = resid.tile([128, TQ], F32, tag="sq", bufs=2,
                             name=f"mu2_{lbl}")
            nc.vector.tensor_mul(mu2[0:1, :], mu, mu)
            nc.vector.tensor_sub(msq, msq, mu2[0:1, :])  # msq <- var
            nc.scalar.activation(out=msq, in_=msq, func=AF.Sqrt, bias=eps_t,
                                 scale=1.0)              # msq <- std
            ps_mu = ps_tile("av", 0, name=f"pmu_{lbl}")
            nc.tensor.matmul(ps_mu, lhsT=ones_row, rhs=mu,
                             start=True, stop=True)
            rstd = resid.tile([1, TQ], F32, tag="stat", bufs=2,
                              name=f"rstd_{lbl}")
            nc.vector.reciprocal(rstd, msq)
            ps_rstd = ps_tile("t", 0, name=f"prstd_{lbl}")
            nc.tensor.matmul(ps_rstd, lhsT=ones_row, rhs=rstd,
                             start=True, stop=True)
            return ps_mu, ps_rstd

        def ln_apply(stats, x_in, out_t, g_t, be_t, lbl, out_f32=False,
                     dma_out=None):
            """out_t <- LN(x_in)*g+be.  out_f32: final LN (DVE op3, fp32 out);
            else op3 on Pool with bf16 out."""
            ps_mu, ps_rstd = stats
            for fc in range(FT):
                tmp = resid.tile([128, TQ], F32, tag="sq", bufs=2,
                                 name=f"t_{lbl}_{fc}")
                nc.vector.tensor_sub(tmp, x_in[:, fc, :], ps_mu)
                tmp2 = resid.tile([128, TQ], BF16 if not out_f32 else F32,
                                  tag="sqb" if not out_f32 else "sq",
                                  bufs=3 if not out_f32 else 2,
                                  name=f"u_{lbl}_{fc}")
                nc.vector.tensor_mul(tmp2, tmp, ps_rstd)
                eng = nc.vector if out_f32 else nc.gpsimd
                eng.tensor_scalar(out=out_t[:, fc, :], in0=tmp2,
                                  scalar1=g_t[:, fc:fc + 1],
                                  scalar2=be_t[:, fc:fc + 1],
                                  op0=ALU.mult, op1=ALU.add)
                if dma_out is not None:
                    nc.sync.dma_start(out=dma_out[:, fc, :],
                                      in_=out_t[:, fc, :])

        def mk_planes(xf, xp, lbl):
            """xf (bf16 [128,FT,TQ]) -> xp fp8 planes (lo,hi) [128,FT,2,TQ]"""
            for fc in range(FT):
                nc.scalar.activation(out=xp[:, fc, 1, :], in_=xf[:, fc, :],
                                     func=AF.Copy, scale=SX)
                nc.vector.scalar_tensor_tensor(
                    out=xp[:, fc, 0, :], in0=xf[:, fc, :], scalar=SX,
                    in1=xp[:, fc, 1, :], op0=ALU.mult, op1=ALU.subtract)

        # =========== program ===========
        QT = glob.tile([128, FT, 2, TQ], F8, tag="qt", bufs=1, name="QT_s")
        KT = glob.tile([128, FT, KV], F8, tag="kt", bufs=2, name="KT_s")
        attnT = glob.tile([128, FT, TQ], BF16, tag="attnT", bufs=1,
                          name="attnT_s")
        V_s = []
        x_res = resid.tile([128, FT, TQ], F32, tag="res", bufs=2)
        x1p = resid.tile([128, FT, TQ], F32, tag="res", bufs=2, name="x1p")
        with ExitStack() as S1:
            wp = S1.enter_context(tc.tile_pool(name="wself", bufs=1))
            xq_b = wp.tile([128, FT, 2, TQ], F8, tag="xq", bufs=1)
            wq = glob.tile([128, FT, D], F8, tag="wstream", bufs=2,
                           name="wq_s")
            for dc in range(FT):   # per-chunk loads so compute starts early
                nc.sync.dma_start(out=xq_b[:, dc, :, :], in_=xq_d[:, dc, :, :])
                nc.sync.dma_start(out=wq[:, dc, :], in_=w_d["wq_s"][:, dc, :])
            # small consts: packed DMAs on the gpsimd queue
            smallf = const.tile([128, 10 * FT + HT + 32], F32, name="c_small")
            nc.gpsimd.dma_start(out=smallf, in_=smallf_d)
            qrow_sb = const.tile([1, 2 * D], BF16, name="c_qrow")
            nc.gpsimd.dma_start(out=qrow_sb, in_=qrow_d)
            dmb_sb = const.tile([128, 2, 256], F32, name="c_dmb")
            nc.gpsimd.dma_start(out=dmb_sb, in_=dmb_d)
            b_sb = {nm: smallf[:, i * FT:(i + 1) * FT]
                    for i, nm in enumerate(BIAS_NAMES)}
            b1_sb = smallf[:, 10 * FT:10 * FT + HT]
            biasa_sb = smallf[:, 10 * FT + HT:10 * FT + HT + 8]
            biasb_sb = smallf[:, 10 * FT + HT + 8:10 * FT + HT + 16]
            biasa_dve = smallf[:, 10 * FT + HT + 16:10 * FT + HT + 24]
            biasb_dve = smallf[:, 10 * FT + HT + 24:10 * FT + HT + 32]

            xp0 = glob.tile([128, FT, 2, 256], F8, tag="encp", bufs=2,
                            name="xkv_0")
            nc.sync.dma_start(out=xp0, in_=xkv_d[0])
            proj_q8(QT, wq, xq_b, 0, "s")
            wv = wtile8("wv_s", w_d["wv_s"])
            wk = wtile8("wk_s", w_d["wk_s"])
            proj_kv_seg8(KT, V_s, 0, xp0, wk, wv, "v", planes=(1,))
            nc.sync.dma_start(out=x_res, in_=xres_d)
            for seg in range(1, 5):
                xp = glob.tile([128, FT, 2, 256], F8, tag="encp", bufs=2,
                               name=f"xkv_{seg}")
                nc.sync.dma_start(out=xp, in_=xkv_d[seg])
                proj_kv_seg8(KT, V_s, seg, xp, wk, wv, "v", planes=(1,))
            # chunk-A attention only needs kv blocks 0..4 (segs 0..4)
            attn_chunk(QT, KT, V_s, attnT, b_sb["bv_s"], "A", 0, BLOCKS_A,
                       biasa_sb, biasa_dve, {3}, 0)
            for seg in range(5, NSEG):
                xp = glob.tile([128, FT, 2, 256], F8, tag="encp", bufs=2,
                               name=f"xkv_{seg}")
                nc.sync.dma_start(out=xp, in_=xkv_d[seg])
                proj_kv_seg8(KT, V_s, seg, xp, wk, wv, "v", planes=(1,))

        # cross K/V projection interleaved under the Act-bound chunk-B
        # attention: one half-segment of PE work per head.
        KT_c = glob.tile([128, FT, KV], F8, tag="kt", bufs=2, name="KT_c")
        V_c = []
        wvc = wtile8("wv_c", w_d["wv_c"])
        wkc = wtile8("wk_c", w_d["wk_c"])
        _cstate = {"seg": 0, "queue": []}

        def cross_kv_filler():
            if not _cstate["queue"]:
                seg = _cstate["seg"]
                if seg >= NSEG:
                    return
                _cstate["seg"] += 1
                ep = glob.tile([128, FT, 2, 256], F8, tag="encp", bufs=2,
                               name=f"enc_{seg}")
                nc.sync.dma_start(out=ep, in_=enc_d[seg])
                _cstate["queue"] = _kv_units(KT_c, V_c, seg, ep, wkc, wvc,
                                             "vc")
            _cstate["queue"].pop(0)()

        attn_chunk(QT, KT, V_s, attnT, b_sb["bv_s"], "B", CHUNK, BLOCKS_B,
                   biasb_sb, biasb_dve, {3, 5, 6}, 1, filler=cross_kv_filler)
        while _cstate["seg"] < NSEG or _cstate["queue"]:
            cross_kv_filler()
        wo_resid(attnT, wo_s_d, b_sb["bo_s"], x_res, x1p, "s")

        st1 = ln_stats(x1p, "ln1")
        x1f = resid.tile([128, FT, TQ], BF16, tag="resb", bufs=2, name="x1f")
        ln_apply(st1, x1p, x1f, b_sb["g1"], b_sb["be1"], "ln1")
        x1n = resid.tile([128, FT, TQ], F8, tag="xn", bufs=1, name="x1n")
        for fc in range(FT):
            nc.scalar.activation(out=x1n[:, fc, :], in_=x1f[:, fc, :],
                                 func=AF.Copy, scale=SX)
        QT_c = glob.tile([128, FT, 2, TQ], F8, tag="qt", bufs=1, name="QT_c")
        proj_q8(QT_c, wqc_d, x1n, D, "c", whl=True)

        attnT_c = glob.tile([128, FT, TQ], BF16, tag="attnT", bufs=1,
                            name="attnT_c")
        x2p = resid.tile([128, FT, TQ], F32, tag="res", bufs=2, name="x2p")
        attn_cross(QT_c, KT_c, V_c, attnT_c, b_sb["bv_c"])
        wo_resid(attnT_c, wo_c_d, b_sb["bo_c"], x1f, x2p, "c")
        st2 = ln_stats(x2p, "ln2")
        x2fb = resid.tile([128, FT, TQ], BF16, tag="resb", bufs=2,
                          name="x2fb")
        ln_apply(st2, x2p, x2fb, b_sb["g2"], b_sb["be2"], "ln2")

        glob_ctx.close()

        # ---- FFN + LN3 + output ----
        x3 = resid.tile([128, FT, TQ], F32, tag="res", bufs=2, name="x3")
        out_sb = resid.tile([128, FT, TQ], F32, tag="res", bufs=2,
                            name="out_sb")
        with ExitStack() as S5:
            fp5 = S5.enter_context(tc.tile_pool(name="ffn", bufs=1))
            x2n = fp5.tile([128, FT, 2, TQ], F8, tag="x2n", bufs=1,
                           name="x2n")
            mk_planes(x2fb, x2n, "x2n")
            hp = fp5.tile([128, HT, 2, TQ], F8, tag="h", bufs=1, name="hp")
            # stream W1 hi/lo in pieces (small first pieces so the first
            # matmul starts as early as possible after SBUF frees up)
            pieces = [2, 2, 4, 8, 8, 8]          # f-tiles per piece
            ht = 0
            for g, npc in enumerate(pieces):
                w1p = fp5.tile([128, 2, FT, npc * 128], F8, tag="w1", bufs=2,
                               padded_shape=[128, 2, FT, 1024], name=f"w1_{g}")
                for p in range(2):
                    nc.sync.dma_start(
                        out=w1p[:, p, :, :],
                        in_=w1_d[:, p, :, ht * 128:(ht + npc) * 128])
                for i in range(npc):
                    ps = ps_tile("big" if ht % 2 == 0 else "st", 0,
                                 name=f"pf1_{ht}")
                    fsl = slice(i * 128, (i + 1) * 128)
                    for dcp in range(4):         # hi*hi k-packed
                        nc.tensor.matmul(
                            ps, lhsT=w1p[:, 0, 2 * dcp:2 * dcp + 2, fsl],
                            rhs=x2n[:, 2 * dcp:2 * dcp + 2, 1, :],
                            start=(dcp == 0), stop=False, perf_mode=DR)
                    for dc in range(FT):         # cross terms
                        nc.tensor.matmul(
                            ps, lhsT=w1p[:, :, dc, fsl],
                            rhs=x2n[:, dc, :, :],
                            start=False, stop=(dc == FT - 1), perf_mode=DR)
                    hbf = fp5.tile([128, TQ], BF16, tag="hbf", bufs=3,
                                   name=f"hbf_{ht}")
                    nc.scalar.activation(out=hbf, in_=ps, func=AF.Relu,
                                         scale=1.0 / PSC,
                                         bias=b1_sb[:, ht:ht + 1])
                    nc.vector.tensor_scalar_mul(out=hp[:, ht, 1, :],
                                                in0=hbf, scalar1=SH)
                    nc.vector.scalar_tensor_tensor(
                        out=hp[:, ht, 0, :], in0=hbf, scalar=SH,
                        in1=hp[:, ht, 1, :], op0=ALU.mult, op1=ALU.subtract)
                    ht += 1
            # W2 hi/lo: ht-outer with 8 concurrent PSUM accumulators.
            ps8 = [ps_tile(TAG8[fo], 0, name=f"pf2_{fo}") for fo in range(FT)]
            for g in range(4):
                w2p = fp5.tile([128, 2, 8, D], F8, tag="w2p", bufs=2,
                               name=f"w2_{g}")
                for p in range(2):
                    nc.sync.dma_start(out=w2p[:, p, :, :],
                                      in_=w2_d[:, p, g * 8:(g + 1) * 8, :])
                for i in range(0, 8, 2):
                    ht = g * 8 + i
                    for fo in range(FT):
                        fsl = slice(fo * 128, (fo + 1) * 128)
                        nc.tensor.matmul(       # hi*hi k-pair
                            ps8[fo], lhsT=w2p[:, 0, i:i + 2, fsl],
                            rhs=hp[:, ht:ht + 2, 1, :],
                            start=(ht == 0), stop=False, perf_mode=DR)
                        for ii in range(2):     # cross terms
                            nc.tensor.matmul(
                                ps8[fo], lhsT=w2p[:, :, i + ii, fsl],
                                rhs=hp[:, ht + ii, :, :],
                                start=False,
                                stop=(ht + ii == HT - 1), perf_mode=DR)
            for fo in range(FT):
                nc.vector.scalar_tensor_tensor(
                    out=x3[:, fo, :], in0=ps8[fo], scalar=1.0 / (SH * SW),
                    in1=x2fb[:, fo, :], op0=ALU.mult, op1=ALU.add)
            st3 = ln_stats(x3, "ln3")
            ln_apply(st3, x3, out_sb, b_sb["g3"], b_sb["be3"], "ln3",
                     out_f32=True, dma_out=out_d)

    nc.compile()
    return nc


def _to_tiles(a2d, dt=NPBF):
    """[P*128, F] -> [128, P, F] (SBUF tile layout), casting to dt."""
    p8, f = a2d.shape
    return np.ascontiguousarray(
        a2d.reshape(p8 // 128, 128, f).transpose(1, 0, 2).astype(dt))


def _planes8(a2d, scale=SX):
    """[D, F] f32 -> [128, FT, 2, F] fp8 planes (lo, hi), scaled."""
    s = a2d.astype(np.float32) * scale
    hi = s.astype(NP8)
    lo = (s - hi.astype(np.float32)).astype(NP8)
    hi_t = _to_tiles(hi, NP8)
    lo_t = _to_tiles(lo, NP8)
    return np.ascontiguousarray(np.stack([lo_t, hi_t], axis=2))


def _wplanes8(w2d, scale=SW):
    """[D, F] f32 -> [128, 2, FT, F] fp8 planes (hi, lo), scaled."""
    s = w2d.astype(np.float32) * scale
    hi = s.astype(NP8)
    lo = (s - hi.astype(np.float32)).astype(NP8)
    return np.ascontiguousarray(
        np.stack([_to_tiles(hi, NP8), _to_tiles(lo, NP8)], axis=1))


def _w8(w2d, scale=SW):
    return _to_tiles((w2d.astype(np.float32) * scale).astype(NP8), NP8)


def _seg_planes(a2d):
    """[1024, NSEG*256] -> [NSEG, 128, FT, 2, 256] fp8 (seg-major)."""
    segs = [_planes8(a2d[:, s * 256:(s + 1) * 256]) for s in range(NSEG)]
    return np.ascontiguousarray(np.stack(segs))


def _vec_tiles(v, dt=np.float32):
    """[n*128] -> [128, n]"""
    return np.ascontiguousarray(v.reshape(-1, 128).T.astype(dt))


def _prep_core(c, dec, enc, consts):
    j = c % 4
    b = c // 4
    ja, jb = j, 7 - j
    rest = [ch for ch in range(0, jb) if ch != ja]
    qtok = np.r_[ja * CHUNK:(ja + 1) * CHUNK, jb * CHUNK:(jb + 1) * CHUNK]
    kvtok = np.concatenate(
        [qtok] + [np.arange(ch * CHUNK, (ch + 1) * CHUNK) for ch in rest])
    xq = dec[b][qtok]                       # [512, D]
    xkv = np.zeros((KV, D), np.float32)
    xkv[: len(kvtok)] = dec[b][kvtok]
    real_blocks = len(kvtok) // CHUNK

    # per-256-block additive exp biases (0 = attend, NEG = masked)
    biasa = np.full(8, NEG, np.float32)
    biasa[0] = 0.0                          # own diagonal block
    biasa[2:2 + ja] = 0.0                   # prior chunks in the window
    biasb = np.full(8, NEG, np.float32)
    biasb[:real_blocks] = 0.0

    m = dict(consts)
    m["xq"] = _planes8(xq.T)
    m["xres"] = _to_tiles(xq.T, np.float32)
    m["xkv"] = _seg_planes(xkv.T)
    m["enc"] = _seg_planes(enc[b].T)
    m["smallf"] = np.ascontiguousarray(np.concatenate(
        [m.pop("smallf_base"),
         np.repeat(biasa[None, :], 128, axis=0),
         np.repeat(biasb[None, :], 128, axis=0),
         np.repeat(EXPA * biasa[None, :] + EXPB, 128, axis=0),
         np.repeat(EXPA * biasb[None, :] + EXPB, 128, axis=0)],
        axis=1, dtype=np.float32))
    return m, (b, qtok)


def _prep_consts(inputs):
    c = {}
    for src, dst in (("Wq_s", "wq_s"), ("Wk_s", "wk_s"), ("Wv_s", "wv_s"),
                     ("Wk_c", "wk_c"), ("Wv_c", "wv_c")):
        w = np.asarray(inputs[src], np.float32)           # [H, D, DK]
        c[dst] = _w8(w.transpose(1, 0, 2).reshape(D, D))
    c["wq_c"] = _wplanes8(
        np.asarray(inputs["Wq_c"], np.float32).transpose(1, 0, 2).reshape(D, D))
    c["wo_s"] = _to_tiles(np.asarray(inputs["Wo_s"], np.float32))
    c["wo_c"] = _to_tiles(np.asarray(inputs["Wo_c"], np.float32))
    W1 = np.asarray(inputs["W1"], np.float32)
    W2 = np.asarray(inputs["W2"], np.float32)
    b2 = np.asarray(inputs["b2"], np.float32)
    c["w1"] = _wplanes8(W1)
    c["w2"] = _wplanes8(W2)
    b1p = np.asarray(inputs["b1"], np.float32) - b2 @ W1
    be2p = np.asarray(inputs["be2"], np.float32) + b2

    smalls = [_vec_tiles(np.asarray(inputs[nm], np.float32).reshape(-1))
              for nm in ("bo_s", "bo_c", "bv_s", "bv_c", "g1", "be1", "g2")]
    smalls.append(_vec_tiles(be2p))
    smalls += [_vec_tiles(np.asarray(inputs[nm], np.float32).reshape(-1))
               for nm in ("g3", "be3")]
    smalls.append(_vec_tiles(b1p))
    c["smallf_base"] = np.concatenate(smalls, axis=1)
    qrow = np.concatenate(
        [np.asarray(inputs["bq_s"], np.float32).reshape(-1) * PSC,
         np.asarray(inputs["bq_c"], np.float32).reshape(-1) * PSC])
    c["qrow"] = np.ascontiguousarray(qrow[None, :].astype(NPBF))
    # causal diag bias dmb[s_local, half, q] = 0 if (half*128+s)<=q else -1e5
    sg = (np.arange(2)[None, :, None] * 128 + np.arange(128)[:, None, None])
    M = (sg <= np.arange(CHUNK)[None, None, :])
    c["dmb"] = np.where(M, 0.0, -1e5).astype(np.float32)
    return c


def _make_runner(nc):
    """Build the shard_map-jitted executable ONCE (run_bass_kernel_spmd
    re-traces and re-lowers per call, which costs seconds of host time)."""
    import jax
    import concourse.mybir as mybir_
    from concourse import bass2jax
    from jax.experimental.shard_map import shard_map
    from jax.sharding import Mesh, PartitionSpec

    bass2jax.install_neuronx_cc_hook()
    part_name = (nc.partition_id_tensor.name if nc.partition_id_tensor
                 else None)
    in_names, out_names, out_avals, zero_outs = [], [], [], []
    for alloc in nc.m.functions[0].allocations:
        if not isinstance(alloc, mybir_.MemoryLocationSet):
            continue
        name = alloc.memorylocations[0].name
        if alloc.kind == "ExternalInput":
            if name != part_name:
                in_names.append(name)
        elif alloc.kind == "ExternalOutput":
            shape = tuple(alloc.tensor_shape)
            dtype = mybir_.dt.np(alloc.dtype)
            out_names.append(name)
            out_avals.append(jax.core.ShapedArray(shape, dtype))
            zero_outs.append(np.zeros(shape, dtype))
    n_params = len(in_names)
    all_names = in_names + out_names
    if part_name is not None:
        all_names = all_names + [part_name]
    donate = tuple(range(n_params, n_params + len(out_names)))

    def _body(*args):
        operands = list(args)
        if part_name is not None:
            operands.append(bass2jax.partition_id_tensor())
        outs = bass2jax._bass_exec_p.bind(
            *operands, out_avals=tuple(out_avals), in_names=tuple(all_names),
            out_names=tuple(out_names), lowering_input_output_aliases=(),
            sim_require_finite=True, sim_require_nnan=True, nc=nc)
        return tuple(outs)

    # inputs identical on every core are passed replicated (uploaded once)
    REPL = {"wq_s", "wk_s", "wv_s", "wk_c", "wv_c", "wq_c", "wo_s", "wo_c",
            "w1", "w2", "qrow", "dmb"}
    in_specs = tuple(PartitionSpec() if nm in REPL else PartitionSpec("core")
                     for nm in in_names) + \
        (PartitionSpec("core"),) * len(out_names)
    devices = jax.devices()[:N_CORES]
    mesh = Mesh(np.asarray(devices), ("core",))
    sharded = jax.jit(
        shard_map(_body, mesh=mesh, in_specs=in_specs,
                  out_specs=(PartitionSpec("core"),) * len(out_names),
                  check_rep=False),
        donate_argnums=donate, keep_unused=True)

    def run(in_maps):
        concat_in = [
            in_maps[0][nm] if nm in REPL else
            np.concatenate([in_maps[c][nm] for c in range(N_CORES)], axis=0)
            for nm in in_names]
        concat_zero = [
            np.zeros((N_CORES * z.shape[0], *z.shape[1:]), z.dtype)
            for z in zero_outs]
        out_arrs = sharded(*concat_in, *concat_zero)
        return [
            {nm: np.asarray(out_arrs[i]).reshape(N_CORES, *out_avals[i].shape)[c]
             for i, nm in enumerate(out_names)}
            for c in range(N_CORES)]

    return run


def kernel(**inputs):
    global _BUILT, _NC
    if _BUILT is None:
        nc = _NC = _build()
        try:
            from concourse._compat import axon_active
            under_axon = axon_active()
        except ImportError:
            under_axon = False
        if under_axon:
            _BUILT = _make_runner(nc)
        else:
            def _native_run(in_maps, _nc=nc):
                res = run_bass_kernel_spmd(_nc, in_maps,
                                           core_ids=list(range(N_CORES)))
                return res.results
            _BUILT = _native_run
    run = _BUILT

    dec = np.asarray(inputs["dec_input"], np.float32)
    enc = np.asarray(inputs["enc_output"], np.float32)
    consts = _prep_consts(inputs)
    in_maps = []
    metas = []
    for cix in range(N_CORES):
        m, meta = _prep_core(cix, dec, enc, consts)
        in_maps.append(m)
        metas.append(meta)

    results = run(in_maps)

    out = np.empty((B, T, D), np.float32)
    for cix in range(N_CORES):
        b, qtok = metas[cix]
        tiles = results[cix]["out"]           # [128, FT, TQ]
        core_t = tiles.transpose(1, 0, 2).reshape(D, TQ)
        out[b, qtok, :] = core_t.T
    return out


# revision 37
# speedup vs baseline: 1.0147x; 1.0147x over previous
"""Transformer decoder layer (self-attn + cross-attn + FFN, post-LN) on 8
Trainium2 NeuronCores, sequence-parallel with zero collectives, with fp8
DoubleRow matmuls for the large GEMMs.

Sharding: core c -> batch b = c//4, causal-balanced chunk pair (j, 7-j) of
256 tokens each (j = c%4), so every core owns 512 query tokens with equal
total causal attention area. Weights are replicated; K/V projections are
recomputed per core. All per-core differences are expressed through input
DATA (token reordering + additive exp-bias masks), so a single SPMD program
serves all 8 cores.

Quantization (validated to rel-err ~1e-2 vs the 2e-2 gate):
- Q/K/V projections (self+cross): W single-fp8 x X hi/lo-fp8, DoubleRow
  k-packed (contract 256/instr at 0.5 cyc/row) -> 2x PE vs bf16.
- cross-Q: W hi/lo x x1n single-fp8 (2-term).
- scores: K single-fp8 (stride-0 slot broadcast) x Q exact (hi+lo slots),
  one DoubleRow instr per s-tile -> 2x.
- attn*V: exp output directly fp8, V fp8, DoubleRow s-tile pairs -> 4x.
- W_O self/cross: bf16 (attnT unquantized, saves error budget).
- FFN: 3-term hi/lo x hi/lo (drop lo*lo) -> 1.33x.
Scales (powers of 2, exact): X*8, W*64, V*4, h*8; all folded into
activation scales / copy scalars; K-bias dropped (softmax-invariant),
V-bias folded into the attnT write, b2 folded into LN2's beta with
b1' = b1 - b2 @ W1 compensating in the FFN.
"""

import sys

if "/opt/trn_rl_repo" not in sys.path:
    sys.path.insert(0, "/opt/trn_rl_repo")

from contextlib import ExitStack

import numpy as np
import ml_dtypes

import concourse.bass as bass
import concourse.bacc as bacc
import concourse.tile as tile
import concourse.mybir as mybir
from concourse.bass_utils import run_bass_kernel_spmd
from concourse.masks import make_identity

F32 = mybir.dt.float32
BF16 = mybir.dt.bfloat16
F8 = mybir.dt.float8e4
U8 = mybir.dt.uint8
AF = mybir.ActivationFunctionType
ALU = mybir.AluOpType
DR = mybir.MatmulPerfMode.DoubleRow

NP8 = ml_dtypes.float8_e4m3
NPBF = ml_dtypes.bfloat16

D = 1024
H = 16
DK = 64
DFF = 4096
B = 2
T = 2048
N_CORES = 8
CHUNK = 256
TQ = 512          # query tokens per core
KV = 2048         # padded kv layout length (self), enc length (cross)
FT = D // 128     # 8 f-tiles
HT = DFF // 128   # 32 ffn tiles
NSEG = 8          # kv/enc DMA-streaming segments of 256 tokens
NEG = -50.0       # additive pre-exp mask (exp(-50) ~ 2e-22)

SX = 8.0          # activation fp8 scale
SW = 64.0         # weight fp8 scale
SV = 4.0          # V fp8 scale
SH = 8.0          # ffn hidden fp8 scale
PSC = SX * SW     # psum scale of an X*W product (512)
EXPA = float(8.0 / np.log(2.0))   # fp8-Schraudolph exp slope
EXPB = 55.5                       # fp8-Schraudolph exp offset (e4m3 bias)

# self-attn 256-token s-block schedules over the kv layout
# [A(256) | B(256) | rest... | pad]  (block = 2 s-tiles of 128):
BLOCKS_A = [0, 2, 3, 4]    # own diag + 768-token prior window
BLOCKS_B = list(range(8))  # everything (pads masked via bias)

_BUILT = None
_NC = None


def _build():
    nc = bacc.Bacc("TRN2", target_bir_lowering=False, debug=False,
                   num_devices=N_CORES)

    def din(name, shape, dt):
        return nc.dram_tensor(name, shape, dt, kind="ExternalInput").ap()

    xq_d = din("xq", [128, FT, 2, TQ], F8)          # planes (lo, hi), *SX
    xres_d = din("xres", [128, FT, TQ], F32)
    xkv_d = din("xkv", [NSEG, 128, FT, 2, 256], F8)  # seg-major, *SX
    enc_d = din("enc", [NSEG, 128, FT, 2, 256], F8)
    w_d = {}
    for nm in ("wq_s", "wk_s", "wv_s", "wk_c", "wv_c"):
        w_d[nm] = din(nm, [128, FT, D], F8)          # *SW
    wqc_d = din("wq_c", [128, 2, FT, D], F8)         # planes (hi, lo), *SW
    wo_s_d = din("wo_s", [128, FT, D], BF16)
    wo_c_d = din("wo_c", [128, FT, D], BF16)
    w1_d = din("w1", [128, 2, FT, DFF], F8)          # planes (hi, lo), *SW
    w2_d = din("w2", [128, 2, HT, D], F8)            # planes (hi, lo), *SW
    # small fp32 consts packed into one tensor:
    # bo_s(8) bo_c(8) bv_s(8) bv_c(8) g1 be1 g2 be2'(incl b2) g3 be3 (48)
    # b1'(32) biasa(8) biasb(8)  -> 128 columns
    BIAS_NAMES = ("bo_s", "bo_c", "bv_s", "bv_c",
                  "g1", "be1", "g2", "be2", "g3", "be3")
    smallf_d = din("smallf", [128, 10 * FT + HT + 32], F32)
    qrow_d = din("qrow", [1, 2 * D], BF16)           # bq_s*PSC, bq_c*PSC
    dmb_d = din("dmb", [128, 2, 256], F32)           # 0 / -1e5 causal bias
    out_d = nc.dram_tensor("out", [128, FT, TQ], F32, kind="ExternalOutput").ap()

    with tile.TileContext(nc) as tc, ExitStack() as S:
        const = S.enter_context(tc.tile_pool(name="const", bufs=1))
        pp = S.enter_context(tc.tile_pool(name="ps", bufs=1, space="PSUM"))
        resid = S.enter_context(tc.tile_pool(name="resid", bufs=1))

        ident = const.tile([128, 128], BF16)
        make_identity(nc, ident)
        ones_b = const.tile([128, 1], BF16)
        nc.vector.memset(ones_b, 1.0)
        ones_row = const.tile([1, 128], F32)
        nc.vector.memset(ones_row, 1.0)
        ones_q = const.tile([1, TQ], BF16)
        nc.vector.memset(ones_q, 1.0)
        eps_t = const.tile([1, 1], F32)
        nc.vector.memset(eps_t, 1e-5)

        glob_ctx = ExitStack()
        glob = glob_ctx.enter_context(tc.tile_pool(name="glob", bufs=1))

        # =========== helpers ===========
        PS_BUFS = {"big": 2, "st": 3, "av": 2, "t": 1}

        def ps_tile(tag, bufs, shape=(128, 512), dt=F32, name="ps"):
            return pp.tile(list(shape), dt, tag=tag, bufs=PS_BUFS[tag],
                           name=name)

        def wtile8(nm, dram):
            t = glob.tile([128, FT, D], F8, tag="wstream", bufs=2, name=nm)
            for dc in range(FT):     # per-chunk so first consumers start early
                nc.sync.dma_start(out=t[:, dc, :], in_=dram[:, dc, :])
            return t

        TAG8 = ["big", "big", "st", "st", "st", "av", "av", "t"]

        def proj_q8(QT, W_sb, X_sb, qoff, lbl, whl=False):
            """Q projection -> QT [128, FT, 2, TQ] fp8 planes (lo, hi).
            whl=False: W single fp8, X_sb planes (lo,hi) [128,FT,2,TQ].
            whl=True:  W planes (hi,lo) [128,2,FT,D], X single [128,FT,TQ].
            Adds bq via a bf16 bias-row matmul."""
            ps8 = [ps_tile(TAG8[ft], 0, name=f"pjq_{lbl}_{ft}")
                   for ft in range(FT)]
            for ft in range(FT):
                nc.tensor.matmul(
                    ps8[ft], lhsT=qrow_sb[0:1, qoff + ft * 128:
                                          qoff + (ft + 1) * 128],
                    rhs=ones_q, start=True, stop=False)
            for p in range(2):
                for dcp in range(4):
                    if whl:   # stream the W plane/dc-pair piece
                        wqp = glob.tile([128, 2, D], F8, tag="wqc", bufs=2,
                                        name=f"wqp_{p}_{dcp}")
                        nc.sync.dma_start(
                            out=wqp, in_=W_sb[:, p, 2 * dcp:2 * dcp + 2, :])
                    for ft in range(FT):
                        if whl:
                            lhsT = wqp[:, :, ft * 128:(ft + 1) * 128]
                            rhs = X_sb[:, 2 * dcp:2 * dcp + 2, :]
                        else:
                            lhsT = W_sb[:, 2 * dcp:2 * dcp + 2,
                                        ft * 128:(ft + 1) * 128]
                            rhs = X_sb[:, 2 * dcp:2 * dcp + 2, p, :]
                        nc.tensor.matmul(
                            ps8[ft], lhsT=lhsT, rhs=rhs, start=False,
                            stop=(p == 1 and dcp == 3), perf_mode=DR)
            for ft in range(FT):
                # hi plane on Act (psum scale PSC -> *SX), lo residual on DVE
                nc.scalar.activation(out=QT[:, ft, 1, :], in_=ps8[ft],
                                     func=AF.Copy, scale=SX / PSC)
                nc.vector.scalar_tensor_tensor(
                    out=QT[:, ft, 0, :], in0=ps8[ft], scalar=SX / PSC,
                    in1=QT[:, ft, 1, :], op0=ALU.mult, op1=ALU.subtract)

        def _kv_units(KT, V_list, seg, X_piece, WK_sb, WV_sb, vtag,
                      planes=(0, 1)):
            """4 sub-units (2 V-halves, 2 K-halves) for one 256-token seg.
            X_piece [128, FT, 2, 256] planes (lo, hi); planes=(1,) drops the
            lo residual (single-quant X, ~2x fewer matmuls)."""
            sl = slice(seg * 256, (seg + 1) * 256)
            vt = glob.tile([128, 2, H, DK + 1], F8, tag="v", bufs=16,
                           name=f"v_{vtag}_{seg}")
            V_list.append(vt)

            def v_unit(sti):
                for half in range(2):
                    ps = ps_tile("big", 0,
                                 name=f"pv_{vtag}_{seg}_{sti}_{half}")
                    for pi, p in enumerate(planes):
                        for dcp in range(4):
                            nc.tensor.matmul(
                                ps,
                                lhsT=X_piece[:, 2 * dcp:2 * dcp + 2, p,
                                             sti * 128:(sti + 1) * 128],
                                rhs=WV_sb[:, 2 * dcp:2 * dcp + 2,
                                          half * 512:(half + 1) * 512],
                                start=(pi == 0 and dcp == 0),
                                stop=(pi == len(planes) - 1 and dcp == 3),
                                perf_mode=DR)
                    nc.vector.tensor_scalar_mul(
                        out=vt[:, sti, half * 8:(half + 1) * 8, 0:DK],
                        in0=ps.rearrange("p (a b) -> p a b", b=DK),
                        scalar1=SV / PSC)
                if sti == 1:
                    nc.vector.memset(vt[:, :, :, DK:DK + 1], 1.0)

            def k_unit(fpp):
                for ftp in (fpp * 2, fpp * 2 + 1):
                    ps = ps_tile("st", 0, shape=(128, 2, 256),
                                 name=f"pjk_{vtag}_{seg}_{ftp}")
                    for fi in range(2):
                        ft = ftp * 2 + fi
                        for pi, p in enumerate(planes):
                            for dcp in range(4):
                                nc.tensor.matmul(
                                    ps[:, fi, :],
                                    lhsT=WK_sb[:, 2 * dcp:2 * dcp + 2,
                                               ft * 128:(ft + 1) * 128],
                                    rhs=X_piece[:, 2 * dcp:2 * dcp + 2, p, :],
                                    start=(pi == 0 and dcp == 0),
                                    stop=(pi == len(planes) - 1 and dcp == 3),
                                    perf_mode=DR)
                    nc.vector.tensor_scalar_mul(
                        out=KT[:, ftp * 2:ftp * 2 + 2, sl], in0=ps,
                        scalar1=SX / PSC)

            return [lambda: v_unit(0), lambda: v_unit(1),
                    lambda: k_unit(0), lambda: k_unit(1)]

        def proj_kv_seg8(KT, V_list, seg, X_piece, WK_sb, WV_sb, vtag,
                         planes=(0, 1)):
            for u in _kv_units(KT, V_list, seg, X_piece, WK_sb, WV_sb, vtag,
                               planes):
                u()

        # Normalized attention tiles go through a PE transpose whose input
        # comes from a short DVE chain; the transposes are deferred into the
        # NEXT head's PE stream to avoid stalling the in-order PE queue.
        pending_t = []
        _tcnt = [0]

        def _norm1(psav, attnT, po, fp, q0, bv_t, nm):
            rec = glob.tile([128, 1], F32, tag="rec", bufs=8, name=f"r{nm}")
            nc.vector.reciprocal(rec, psav[:, DK:DK + 1])
            an = glob.tile([128, DK], BF16, tag="an", bufs=8, name=f"n{nm}")
            nc.vector.tensor_scalar_mul(an, psav[:, 0:DK], rec)
            pending_t.append((an, attnT, po, fp, q0, bv_t))

        def flush_t():
            """Group the 4 pending (2 heads x 2 qt) tiles per (fp, 256-q
            span) into one PSUM tile and a single biased Act copy."""
            groups = {}
            for an, attnT, po, fp, q0, bv_t in pending_t:
                key = (id(attnT), fp, q0 - q0 % 256)
                groups.setdefault(key, []).append((an, attnT, po, fp, q0,
                                                   bv_t))
            for key, items in groups.items():
                _tcnt[0] += 1
                if len(items) == 4:
                    pst = ps_tile("t", 0, shape=(128, 256), dt=BF16,
                                  name=f"pt{_tcnt[0]}")
                    _, attnT, _, fp, _, bv_t = items[0]
                    qb = key[2]
                    for an, _, po, _, q0, _ in items:
                        nc.tensor.transpose(
                            pst[po:po + DK, q0 - qb:q0 - qb + 128], an, ident)
                    nc.scalar.activation(
                        out=attnT[:, fp, qb:qb + 256], in_=pst,
                        func=AF.Identity, scale=1.0 / SV,
                        bias=bv_t[:, fp:fp + 1])
                else:
                    for an, attnT, po, fp, q0, bv_t in items:
                        pst = ps_tile("t", 0, shape=(DK, 128), dt=BF16,
                                      name=f"pt{_tcnt[0]}")
                        nc.tensor.transpose(pst, an, ident)
                        nc.scalar.activation(
                            out=attnT[po:po + DK, fp, q0:q0 + 128], in_=pst,
                            func=AF.Identity, scale=1.0 / SV,
                            bias=bv_t[po:po + DK, fp:fp + 1])
            pending_t.clear()

        def sc_matmul(ps_out, KT, QT, po, fp, st, qsl, start=True):
            kb = KT[po:po + DK, fp, st * 128:(st + 1) * 128] \
                .unsqueeze(1).broadcast_to((DK, 2, 128))
            nc.tensor.matmul(ps_out, lhsT=kb,
                             rhs=QT[po:po + DK, fp, :, qsl],
                             start=start, stop=True, perf_mode=DR,
                             skip_group_check=not start)

        def attn_chunk(QT, KT, V_list, attnT, bv_t, cn, qoff, blocks, bias2,
                       bias2dve, dveset, diag_blk, filler=None):
            qsl = slice(qoff, qoff + CHUNK)
            for h in range(H):
                fp, po = h // 2, (h % 2) * DK
                ats = {}
                for blk in blocks:
                    ps = ps_tile("st", 0, name=f"pss_{h}_{cn}_{blk}")
                    for half in range(2):
                        sc_matmul(ps[:, half * 256:(half + 1) * 256],
                                  KT, QT, po, fp, blk * 2 + half, qsl)
                    if blk == diag_blk:
                        psv = ps.rearrange("p (a b) -> p a b", b=256)
                        nc.vector.tensor_tensor(out=psv, in0=psv, in1=dmb_sb,
                                                op=ALU.add)
                    dve_exp = blk in dveset
                    at = glob.tile([128, 2, 256], U8 if dve_exp else F8,
                                   tag="at", bufs=10, name=f"a_{h}_{cn}_{blk}")
                    if dve_exp:
                        nc.vector.tensor_scalar(
                            out=at.rearrange("p a b -> p (a b)"), in0=ps,
                            scalar1=EXPA * 0.125 / (SX * SX),
                            scalar2=bias2dve[:, blk:blk + 1],
                            op0=ALU.mult, op1=ALU.add)
                    else:
                        nc.scalar.activation(
                            out=at.rearrange("p a b -> p (a b)"), in_=ps,
                            func=AF.Exp, scale=0.125 / (SX * SX),
                            bias=bias2[:, blk:blk + 1])
                    ats[blk] = at.bitcast(F8) if dve_exp else at
                if h % 2 == 0:
                    flush_t()
                for qt in range(2):
                    psav = ps_tile("av", 0, shape=(128, DK + 1),
                                   name=f"pav_{h}_{cn}_{qt}")
                    for i, blk in enumerate(blocks):
                        nc.tensor.matmul(
                            psav,
                            lhsT=ats[blk][:, :, qt * 128:(qt + 1) * 128],
                            rhs=V_list[blk][:, :, h, :],
                            start=(i == 0), stop=(i == len(blocks) - 1),
                            perf_mode=DR)
                    _norm1(psav, attnT, po, fp, qoff + qt * 128, bv_t,
                           f"s_{h}_{cn}_{qt}")
                if filler is not None:
                    filler()
                    filler()
            flush_t()

        def attn_cross(QT, KT, V_list, attnT, bv_t):
            # two half-passes over s so only 8 exp tiles are live per head
            for h in range(H):
                fp, po = h // 2, (h % 2) * DK
                psavs = [ps_tile("av" if qt % 2 == 0 else "big", 0,
                                 shape=(128, DK + 1), name=f"pavc_{h}_{qt}")
                         for qt in range(4)]
                for half in range(2):
                    ats = {}
                    for pr in range(half * 4, half * 4 + 4):
                        # pairs 1/4/6 compute exp on DVE via the fp8
                        # Schraudolph bit trick (uint8(A*x+B) bitcast to
                        # e4m3); rest stay on the Act engine. Splits the
                        # exp load across both engines.
                        dve_exp = pr in (1, 3, 4, 6)
                        at = glob.tile([128, 2, TQ], U8 if dve_exp else F8,
                                       tag="atc", bufs=5, name=f"ac_{h}_{pr}")
                        for sti in range(2):
                            st = pr * 2 + sti
                            ps = ps_tile("st", 0, name=f"psc_{h}_{st}")
                            sc_matmul(ps, KT, QT, po, fp, st,
                                      slice(0, TQ))
                            if dve_exp:
                                nc.vector.tensor_scalar(
                                    out=at[:, sti, :], in0=ps,
                                    scalar1=EXPA * 0.125 / (SX * SX),
                                    scalar2=EXPB,
                                    op0=ALU.mult, op1=ALU.add)
                            else:
                                nc.scalar.activation(
                                    out=at[:, sti, :], in_=ps, func=AF.Exp,
                                    scale=0.125 / (SX * SX))
                        ats[pr] = at.bitcast(F8) if dve_exp else at
                    if half == 0 and h % 2 == 0:
                        flush_t()
                    for qt in range(4):
                        for pr in range(half * 4, half * 4 + 4):
                            nc.tensor.matmul(
                                psavs[qt],
                                lhsT=ats[pr][:, :, qt * 128:(qt + 1) * 128],
                                rhs=V_list[pr][:, :, h, :],
                                start=(pr == 0), stop=(pr == 7),
                                perf_mode=DR)
                for qt in range(4):
                    _norm1(psavs[qt], attnT, po, fp, qt * 128, bv_t,
                           f"c_{h}_{qt}")
            flush_t()

        def wo_resid(attnT, wo_dram, bo_t, x_prev, x_out, lbl):
            for fp2 in range(4):
                wop = glob.tile([128, FT, 256], BF16, tag="wop", bufs=2,
                                name=f"wo_{lbl}_{fp2}")
                nc.sync.dma_start(
                    out=wop, in_=wo_dram[:, :, fp2 * 256:(fp2 + 1) * 256])
                for f2 in range(2):
                    fo = fp2 * 2 + f2
                    ps = ps_tile("big" if fo % 2 == 0 else "st", 0,
                                 name=f"pwo_{lbl}_{fo}")
                    for fi in range(FT):
                        nc.tensor.matmul(
                            ps, lhsT=wop[:, fi, f2 * 128:(f2 + 1) * 128],
                            rhs=attnT[:, fi, :],
                            start=(fi == 0), stop=(fi == FT - 1))
                    nc.vector.scalar_tensor_tensor(
                        out=x_out[:, fo, :], in0=ps,
                        scalar=bo_t[:, fo:fo + 1],
                        in1=x_prev[:, fo, :], op0=ALU.add, op1=ALU.add)

        def ln_stats(x_in, lbl):
            """-> (ps_mu, ps_rstd) broadcast PSUM tiles."""
            ps_sum = ps_tile("st", 0, shape=(1, TQ), name=f"psum_{lbl}")
            ps_sq = ps_tile("big", 0, shape=(1, TQ), name=f"psq_{lbl}")
            for fc in range(FT):
                xb = resid.tile([128, TQ], BF16, tag="sqb", bufs=3,
                                name=f"xb_{lbl}_{fc}")
                nc.gpsimd.tensor_copy(out=xb, in_=x_in[:, fc, :])
                nc.tensor.matmul(ps_sum, lhsT=ones_b, rhs=xb,
                                 start=(fc == 0), stop=(fc == FT - 1))
                sqb = resid.tile([128, TQ], BF16, tag="sqb", bufs=3,
                                 name=f"sq_{lbl}_{fc}")
                nc.vector.tensor_mul(sqb, xb, xb)
                nc.tensor.matmul(ps_sq, lhsT=ones_b, rhs=sqb,
                                 start=(fc == 0), stop=(fc == FT - 1))
            return ln_finish(ps_sum, ps_sq, lbl)

        def ln_finish(ps_sum, ps_sq, lbl):
            mu = resid.tile([1, TQ], F32, tag="stat", bufs=2, name=f"mu_{lbl}")
            nc.scalar.activation(out=mu, in_=ps_sum, func=AF.Copy, scale=1.0 / D)
            msq = resid.tile([1, TQ], F32, tag="stat", bufs=2,
                             name=f"msq_{lbl}")
            nc.scalar.activation(out=msq, in_=ps_sq, func=AF.Copy, scale=1.0 / D)
            mu2 = resid.tile([128, TQ], F32, tag="sq", bufs=2,
                             name=f"mu2_{lbl}")
            nc.vector.tensor_mul(mu2[0:1, :], mu, mu)
            nc.vector.tensor_sub(msq, msq, mu2[0:1, :])  # msq <- var
            nc.scalar.activation(out=msq, in_=msq, func=AF.Sqrt, bias=eps_t,
                                 scale=1.0)              # msq <- std
            ps_mu = ps_tile("av", 0, name=f"pmu_{lbl}")
            nc.tensor.matmul(ps_mu, lhsT=ones_row, rhs=mu,
                             start=True, stop=True)
            rstd = resid.tile([1, TQ], F32, tag="stat", bufs=2,
                              name=f"rstd_{lbl}")
            nc.vector.reciprocal(rstd, msq)
            ps_rstd = ps_tile("t", 0, name=f"prstd_{lbl}")
            nc.tensor.matmul(ps_rstd, lhsT=ones_row, rhs=rstd,
                             start=True, stop=True)
            return ps_mu, ps_rstd

        def ln_apply(stats, x_in, out_t, g_t, be_t, lbl, out_f32=False,
                     dma_out=None):
            """out_t <- LN(x_in)*g+be.  out_f32: final LN (DVE op3, fp32 out);
            else op3 on Pool with bf16 out."""
            ps_mu, ps_rstd = stats
            for fc in range(FT):
                tmp = resid.tile([128, TQ], F32, tag="sq", bufs=2,
                                 name=f"t_{lbl}_{fc}")
                nc.vector.tensor_sub(tmp, x_in[:, fc, :], ps_mu)
                tmp2 = resid.tile([128, TQ], BF16 if not out_f32 else F32,
                                  tag="sqb" if not out_f32 else "sq",
                                  bufs=3 if not out_f32 else 2,
                                  name=f"u_{lbl}_{fc}")
                nc.vector.tensor_mul(tmp2, tmp, ps_rstd)
                eng = nc.vector if out_f32 else nc.gpsimd
                eng.tensor_scalar(out=out_t[:, fc, :], in0=tmp2,
                                  scalar1=g_t[:, fc:fc + 1],
                                  scalar2=be_t[:, fc:fc + 1],
                                  op0=ALU.mult, op1=ALU.add)
                if dma_out is not None:
                    nc.sync.dma_start(out=dma_out[:, fc, :],
                                      in_=out_t[:, fc, :])

        def mk_planes(xf, xp, lbl):
            """xf (bf16 [128,FT,TQ]) -> xp fp8 planes (lo,hi) [128,FT,2,TQ]"""
            for fc in range(FT):
                nc.scalar.activation(out=xp[:, fc, 1, :], in_=xf[:, fc, :],
                                     func=AF.Copy, scale=SX)
                nc.vector.scalar_tensor_tensor(
                    out=xp[:, fc, 0, :], in0=xf[:, fc, :], scalar=SX,
                    in1=xp[:, fc, 1, :], op0=ALU.mult, op1=ALU.subtract)

        # =========== program ===========
        QT = glob.tile([128, FT, 2, TQ], F8, tag="qt", bufs=1, name="QT_s")
        KT = glob.tile([128, FT, KV], F8, tag="kt", bufs=2, name="KT_s")
        attnT = glob.tile([128, FT, TQ], BF16, tag="attnT", bufs=1,
                          name="attnT_s")
        V_s = []
        x_res = resid.tile([128, FT, TQ], F32, tag="res", bufs=2)
        x1p = resid.tile([128, FT, TQ], F32, tag="res", bufs=2, name="x1p")
        with ExitStack() as S1:
            wp = S1.enter_context(tc.tile_pool(name="wself", bufs=1))
            xq_b = wp.tile([128, FT, 2, TQ], F8, tag="xq", bufs=1)
            wq = glob.tile([128, FT, D], F8, tag="wstream", bufs=2,
                           name="wq_s")
            for dc in range(FT):   # per-chunk loads so compute starts early
                nc.sync.dma_start(out=xq_b[:, dc, :, :], in_=xq_d[:, dc, :, :])
                nc.sync.dma_start(out=wq[:, dc, :], in_=w_d["wq_s"][:, dc, :])
            # small consts: packed DMAs on the gpsimd queue
            smallf = const.tile([128, 10 * FT + HT + 32], F32, name="c_small")
            nc.gpsimd.dma_start(out=smallf, in_=smallf_d)
            qrow_sb = const.tile([1, 2 * D], BF16, name="c_qrow")
            nc.gpsimd.dma_start(out=qrow_sb, in_=qrow_d)
            dmb_sb = const.tile([128, 2, 256], F32, name="c_dmb")
            nc.gpsimd.dma_start(out=dmb_sb, in_=dmb_d)
            b_sb = {nm: smallf[:, i * FT:(i + 1) * FT]
                    for i, nm in enumerate(BIAS_NAMES)}
            b1_sb = smallf[:, 10 * FT:10 * FT + HT]
            biasa_sb = smallf[:, 10 * FT + HT:10 * FT + HT + 8]
            biasb_sb = smallf[:, 10 * FT + HT + 8:10 * FT + HT + 16]
            biasa_dve = smallf[:, 10 * FT + HT + 16:10 * FT + HT + 24]
            biasb_dve = smallf[:, 10 * FT + HT + 24:10 * FT + HT + 32]

            xp0 = glob.tile([128, FT, 2, 256], F8, tag="encp", bufs=2,
                            name="xkv_0")
            nc.sync.dma_start(out=xp0, in_=xkv_d[0])
            proj_q8(QT, wq, xq_b, 0, "s")
            wv = wtile8("wv_s", w_d["wv_s"])
            wk = wtile8("wk_s", w_d["wk_s"])
            proj_kv_seg8(KT, V_s, 0, xp0, wk, wv, "v", planes=(1,))
            nc.sync.dma_start(out=x_res, in_=xres_d)
            for seg in range(1, 5):
                xp = glob.tile([128, FT, 2, 256], F8, tag="encp", bufs=2,
                               name=f"xkv_{seg}")
                nc.sync.dma_start(out=xp, in_=xkv_d[seg])
                proj_kv_seg8(KT, V_s, seg, xp, wk, wv, "v", planes=(1,))
            # chunk-A attention only needs kv blocks 0..4 (segs 0..4)
            attn_chunk(QT, KT, V_s, attnT, b_sb["bv_s"], "A", 0, BLOCKS_A,
                       biasa_sb, biasa_dve, {3}, 0)
            for seg in range(5, NSEG):
                xp = glob.tile([128, FT, 2, 256], F8, tag="encp", bufs=2,
                               name=f"xkv_{seg}")
                nc.sync.dma_start(out=xp, in_=xkv_d[seg])
                proj_kv_seg8(KT, V_s, seg, xp, wk, wv, "v", planes=(1,))

        # cross K/V projection interleaved under the Act-bound chunk-B
        # attention: one half-segment of PE work per head.
        KT_c = glob.tile([128, FT, KV], F8, tag="kt", bufs=2, name="KT_c")
        V_c = []
        wvc = wtile8("wv_c", w_d["wv_c"])
        wkc = wtile8("wk_c", w_d["wk_c"])
        _cstate = {"seg": 0, "queue": []}

        def cross_kv_filler():
            if not _cstate["queue"]:
                seg = _cstate["seg"]
                if seg >= NSEG:
                    return
                _cstate["seg"] += 1
                ep = glob.tile([128, FT, 2, 256], F8, tag="encp", bufs=2,
                               name=f"enc_{seg}")
                nc.sync.dma_start(out=ep, in_=enc_d[seg])
                _cstate["queue"] = _kv_units(KT_c, V_c, seg, ep, wkc, wvc,
                                             "vc")
            _cstate["queue"].pop(0)()

        attn_chunk(QT, KT, V_s, attnT, b_sb["bv_s"], "B", CHUNK, BLOCKS_B,
                   biasb_sb, biasb_dve, {3, 6}, 1, filler=cross_kv_filler)
        while _cstate["seg"] < NSEG or _cstate["queue"]:
            cross_kv_filler()
        wo_resid(attnT, wo_s_d, b_sb["bo_s"], x_res, x1p, "s")

        st1 = ln_stats(x1p, "ln1")
        x1f = resid.tile([128, FT, TQ], BF16, tag="resb", bufs=2, name="x1f")
        ln_apply(st1, x1p, x1f, b_sb["g1"], b_sb["be1"], "ln1")
        x1n = resid.tile([128, FT, TQ], F8, tag="xn", bufs=1, name="x1n")
        for fc in range(FT):
            nc.scalar.activation(out=x1n[:, fc, :], in_=x1f[:, fc, :],
                                 func=AF.Copy, scale=SX)
        QT_c = glob.tile([128, FT, 2, TQ], F8, tag="qt", bufs=1, name="QT_c")
        proj_q8(QT_c, wqc_d, x1n, D, "c", whl=True)

        attnT_c = glob.tile([128, FT, TQ], BF16, tag="attnT", bufs=1,
                            name="attnT_c")
        x2p = resid.tile([128, FT, TQ], F32, tag="res", bufs=2, name="x2p")
        attn_cross(QT_c, KT_c, V_c, attnT_c, b_sb["bv_c"])
        wo_resid(attnT_c, wo_c_d, b_sb["bo_c"], x1f, x2p, "c")
        st2 = ln_stats(x2p, "ln2")
        x2fb = resid.tile([128, FT, TQ], BF16, tag="resb", bufs=2,
                          name="x2fb")
        ln_apply(st2, x2p, x2fb, b_sb["g2"], b_sb["be2"], "ln2")

        glob_ctx.close()

        # ---- FFN + LN3 + output ----
        x3 = resid.tile([128, FT, TQ], F32, tag="res", bufs=2, name="x3")
        out_sb = resid.tile([128, FT, TQ], F32, tag="res", bufs=2,
                            name="out_sb")
        with ExitStack() as S5:
            fp5 = S5.enter_context(tc.tile_pool(name="ffn", bufs=1))
            x2n = fp5.tile([128, FT, 2, TQ], F8, tag="x2n", bufs=1,
                           name="x2n")
            mk_planes(x2fb, x2n, "x2n")
            hp = fp5.tile([128, HT, 2, TQ], F8, tag="h", bufs=1, name="hp")
            # stream W1 hi/lo in pieces (small first pieces so the first
            # matmul starts as early as possible after SBUF frees up)
            pieces = [2, 2, 4, 8, 8, 8]          # f-tiles per piece
            ht = 0
            for g, npc in enumerate(pieces):
                w1p = fp5.tile([128, 2, FT, npc * 128], F8, tag="w1", bufs=2,
                               padded_shape=[128, 2, FT, 1024], name=f"w1_{g}")
                for p in range(2):
                    nc.sync.dma_start(
                        out=w1p[:, p, :, :],
                        in_=w1_d[:, p, :, ht * 128:(ht + npc) * 128])
                for i in range(npc):
                    ps = ps_tile("big" if ht % 2 == 0 else "st", 0,
                                 name=f"pf1_{ht}")
                    fsl = slice(i * 128, (i + 1) * 128)
                    for dcp in range(4):         # hi*hi k-packed
                        nc.tensor.matmul(
                            ps, lhsT=w1p[:, 0, 2 * dcp:2 * dcp + 2, fsl],
                            rhs=x2n[:, 2 * dcp:2 * dcp + 2, 1, :],
                            start=(dcp == 0), stop=False, perf_mode=DR)
                    for dc in range(FT):         # cross terms
                        nc.tensor.matmul(
                            ps, lhsT=w1p[:, :, dc, fsl],
                            rhs=x2n[:, dc, :, :],
                            start=False, stop=(dc == FT - 1), perf_mode=DR)
                    hbf = fp5.tile([128, TQ], BF16, tag="hbf", bufs=3,
                                   name=f"hbf_{ht}")
                    nc.scalar.activation(out=hbf, in_=ps, func=AF.Relu,
                                         scale=1.0 / PSC,
                                         bias=b1_sb[:, ht:ht + 1])
                    nc.vector.tensor_scalar_mul(out=hp[:, ht, 1, :],
                                                in0=hbf, scalar1=SH)
                    nc.vector.scalar_tensor_tensor(
                        out=hp[:, ht, 0, :], in0=hbf, scalar=SH,
                        in1=hp[:, ht, 1, :], op0=ALU.mult, op1=ALU.subtract)
                    ht += 1
            # W2 hi/lo: ht-outer with 8 concurrent PSUM accumulators.
            ps8 = [ps_tile(TAG8[fo], 0, name=f"pf2_{fo}") for fo in range(FT)]
            for g in range(4):
                w2p = fp5.tile([128, 2, 8, D], F8, tag="w2p", bufs=2,
                               name=f"w2_{g}")
                for p in range(2):
                    nc.sync.dma_start(out=w2p[:, p, :, :],
                                      in_=w2_d[:, p, g * 8:(g + 1) * 8, :])
                for i in range(0, 8, 2):
                    ht = g * 8 + i
                    for fo in range(FT):
                        fsl = slice(fo * 128, (fo + 1) * 128)
                        nc.tensor.matmul(       # hi*hi k-pair
                            ps8[fo], lhsT=w2p[:, 0, i:i + 2, fsl],
                            rhs=hp[:, ht:ht + 2, 1, :],
                            start=(ht == 0), stop=False, perf_mode=DR)
                        for ii in range(2):     # cross terms
                            nc.tensor.matmul(
                                ps8[fo], lhsT=w2p[:, :, i + ii, fsl],
                                rhs=hp[:, ht + ii, :, :],
                                start=False,
                                stop=(ht + ii == HT - 1), perf_mode=DR)
            for fo in range(FT):
                nc.vector.scalar_tensor_tensor(
                    out=x3[:, fo, :], in0=ps8[fo], scalar=1.0 / (SH * SW),
                    in1=x2fb[:, fo, :], op0=ALU.mult, op1=ALU.add)
            st3 = ln_stats(x3, "ln3")
            ln_apply(st3, x3, out_sb, b_sb["g3"], b_sb["be3"], "ln3",
                     out_f32=True, dma_out=out_d)

    nc.compile()
    return nc


def _to_tiles(a2d, dt=NPBF):
    """[P*128, F] -> [128, P, F] (SBUF tile layout), casting to dt."""
    p8, f = a2d.shape
    return np.ascontiguousarray(
        a2d.reshape(p8 // 128, 128, f).transpose(1, 0, 2).astype(dt))


def _planes8(a2d, scale=SX):
    """[D, F] f32 -> [128, FT, 2, F] fp8 planes (lo, hi), scaled."""
    s = a2d.astype(np.float32) * scale
    hi = s.astype(NP8)
    lo = (s - hi.astype(np.float32)).astype(NP8)
    hi_t = _to_tiles(hi, NP8)
    lo_t = _to_tiles(lo, NP8)
    return np.ascontiguousarray(np.stack([lo_t, hi_t], axis=2))


def _wplanes8(w2d, scale=SW):
    """[D, F] f32 -> [128, 2, FT, F] fp8 planes (hi, lo), scaled."""
    s = w2d.astype(np.float32) * scale
    hi = s.astype(NP8)
    lo = (s - hi.astype(np.float32)).astype(NP8)
    return np.ascontiguousarray(
        np.stack([_to_tiles(hi, NP8), _to_tiles(lo, NP8)], axis=1))


def _w8(w2d, scale=SW):
    return _to_tiles((w2d.astype(np.float32) * scale).astype(NP8), NP8)


def _seg_planes(a2d):
    """[1024, NSEG*256] -> [NSEG, 128, FT, 2, 256] fp8 (seg-major)."""
    segs = [_planes8(a2d[:, s * 256:(s + 1) * 256]) for s in range(NSEG)]
    return np.ascontiguousarray(np.stack(segs))


def _vec_tiles(v, dt=np.float32):
    """[n*128] -> [128, n]"""
    return np.ascontiguousarray(v.reshape(-1, 128).T.astype(dt))


def _prep_core(c, dec, enc, consts):
    j = c % 4
    b = c // 4
    ja, jb = j, 7 - j
    rest = [ch for ch in range(0, jb) if ch != ja]
    qtok = np.r_[ja * CHUNK:(ja + 1) * CHUNK, jb * CHUNK:(jb + 1) * CHUNK]
    kvtok = np.concatenate(
        [qtok] + [np.arange(ch * CHUNK, (ch + 1) * CHUNK) for ch in rest])
    xq = dec[b][qtok]                       # [512, D]
    xkv = np.zeros((KV, D), np.float32)
    xkv[: len(kvtok)] = dec[b][kvtok]
    real_blocks = len(kvtok) // CHUNK

    # per-256-block additive exp biases (0 = attend, NEG = masked)
    biasa = np.full(8, NEG, np.float32)
    biasa[0] = 0.0                          # own diagonal block
    biasa[2:2 + ja] = 0.0                   # prior chunks in the window
    biasb = np.full(8, NEG, np.float32)
    biasb[:real_blocks] = 0.0

    m = dict(consts)
    m["xq"] = _planes8(xq.T)
    m["xres"] = _to_tiles(xq.T, np.float32)
    m["xkv"] = _seg_planes(xkv.T)
    m["enc"] = _seg_planes(enc[b].T)
    m["smallf"] = np.ascontiguousarray(np.concatenate(
        [m.pop("smallf_base"),
         np.repeat(biasa[None, :], 128, axis=0),
         np.repeat(biasb[None, :], 128, axis=0),
         np.repeat(EXPA * biasa[None, :] + EXPB, 128, axis=0),
         np.repeat(EXPA * biasb[None, :] + EXPB, 128, axis=0)],
        axis=1, dtype=np.float32))
    return m, (b, qtok)


def _prep_consts(inputs):
    c = {}
    for src, dst in (("Wq_s", "wq_s"), ("Wk_s", "wk_s"), ("Wv_s", "wv_s"),
                     ("Wk_c", "wk_c"), ("Wv_c", "wv_c")):
        w = np.asarray(inputs[src], np.float32)           # [H, D, DK]
        c[dst] = _w8(w.transpose(1, 0, 2).reshape(D, D))
    c["wq_c"] = _wplanes8(
        np.asarray(inputs["Wq_c"], np.float32).transpose(1, 0, 2).reshape(D, D))
    c["wo_s"] = _to_tiles(np.asarray(inputs["Wo_s"], np.float32))
    c["wo_c"] = _to_tiles(np.asarray(inputs["Wo_c"], np.float32))
    W1 = np.asarray(inputs["W1"], np.float32)
    W2 = np.asarray(inputs["W2"], np.float32)
    b2 = np.asarray(inputs["b2"], np.float32)
    c["w1"] = _wplanes8(W1)
    c["w2"] = _wplanes8(W2)
    b1p = np.asarray(inputs["b1"], np.float32) - b2 @ W1
    be2p = np.asarray(inputs["be2"], np.float32) + b2

    smalls = [_vec_tiles(np.asarray(inputs[nm], np.float32).reshape(-1))
              for nm in ("bo_s", "bo_c", "bv_s", "bv_c", "g1", "be1", "g2")]
    smalls.append(_vec_tiles(be2p))
    smalls += [_vec_tiles(np.asarray(inputs[nm], np.float32).reshape(-1))
               for nm in ("g3", "be3")]
    smalls.append(_vec_tiles(b1p))
    c["smallf_base"] = np.concatenate(smalls, axis=1)
    qrow = np.concatenate(
        [np.asarray(inputs["bq_s"], np.float32).reshape(-1) * PSC,
         np.asarray(inputs["bq_c"], np.float32).reshape(-1) * PSC])
    c["qrow"] = np.ascontiguousarray(qrow[None, :].astype(NPBF))
    # causal diag bias dmb[s_local, half, q] = 0 if (half*128+s)<=q else -1e5
    sg = (np.arange(2)[None, :, None] * 128 + np.arange(128)[:, None, None])
    M = (sg <= np.arange(CHUNK)[None, None, :])
    c["dmb"] = np.where(M, 0.0, -1e5).astype(np.float32)
    return c


def _make_runner(nc):
    """Build the shard_map-jitted executable ONCE (run_bass_kernel_spmd
    re-traces and re-lowers per call, which costs seconds of host time)."""
    import jax
    import concourse.mybir as mybir_
    from concourse import bass2jax
    from jax.experimental.shard_map import shard_map
    from jax.sharding import Mesh, PartitionSpec

    bass2jax.install_neuronx_cc_hook()
    part_name = (nc.partition_id_tensor.name if nc.partition_id_tensor
                 else None)
    in_names, out_names, out_avals, zero_outs = [], [], [], []
    for alloc in nc.m.functions[0].allocations:
        if not isinstance(alloc, mybir_.MemoryLocationSet):
            continue
        name = alloc.memorylocations[0].name
        if alloc.kind == "ExternalInput":
            if name != part_name:
                in_names.append(name)
        elif alloc.kind == "ExternalOutput":
            shape = tuple(alloc.tensor_shape)
            dtype = mybir_.dt.np(alloc.dtype)
            out_names.append(name)
            out_avals.append(jax.core.ShapedArray(shape, dtype))
            zero_outs.append(np.zeros(shape, dtype))
    n_params = len(in_names)
    all_names = in_names + out_names
    if part_name is not None:
        all_names = all_names + [part_name]
    donate = tuple(range(n_params, n_params + len(out_names)))

    def _body(*args):
        operands = list(args)
        if part_name is not None:
            operands.append(bass2jax.partition_id_tensor())
        outs = bass2jax._bass_exec_p.bind(
            *operands, out_avals=tuple(out_avals), in_names=tuple(all_names),
            out_names=tuple(out_names), lowering_input_output_aliases=(),
            sim_require_finite=True, sim_require_nnan=True, nc=nc)
        return tuple(outs)

    # inputs identical on every core are passed replicated (uploaded once)
    REPL = {"wq_s", "wk_s", "wv_s", "wk_c", "wv_c", "wq_c", "wo_s", "wo_c",
            "w1", "w2", "qrow", "dmb"}
    in_specs = tuple(PartitionSpec() if nm in REPL else PartitionSpec("core")
                     for nm in in_names) + \
        (PartitionSpec("core"),) * len(out_names)
    devices = jax.devices()[:N_CORES]
    mesh = Mesh(np.asarray(devices), ("core",))
    sharded = jax.jit(
        shard_map(_body, mesh=mesh, in_specs=in_specs,
                  out_specs=(PartitionSpec("core"),) * len(out_names),
                  check_rep=False),
        donate_argnums=donate, keep_unused=True)

    def run(in_maps):
        concat_in = [
            in_maps[0][nm] if nm in REPL else
            np.concatenate([in_maps[c][nm] for c in range(N_CORES)], axis=0)
            for nm in in_names]
        concat_zero = [
            np.zeros((N_CORES * z.shape[0], *z.shape[1:]), z.dtype)
            for z in zero_outs]
        out_arrs = sharded(*concat_in, *concat_zero)
        return [
            {nm: np.asarray(out_arrs[i]).reshape(N_CORES, *out_avals[i].shape)[c]
             for i, nm in enumerate(out_names)}
            for c in range(N_CORES)]

    return run


def kernel(**inputs):
    global _BUILT, _NC
    if _BUILT is None:
        nc = _NC = _build()
        try:
            from concourse._compat import axon_active
            under_axon = axon_active()
        except ImportError:
            under_axon = False
        if under_axon:
            _BUILT = _make_runner(nc)
        else:
            def _native_run(in_maps, _nc=nc):
                res = run_bass_kernel_spmd(_nc, in_maps,
                                           core_ids=list(range(N_CORES)))
                return res.results
            _BUILT = _native_run
    run = _BUILT

    dec = np.asarray(inputs["dec_input"], np.float32)
    enc = np.asarray(inputs["enc_output"], np.float32)
    consts = _prep_consts(inputs)
    in_maps = []
    metas = []
    for cix in range(N_CORES):
        m, meta = _prep_core(cix, dec, enc, consts)
        in_maps.append(m)
        metas.append(meta)

    results = run(in_maps)

    out = np.empty((B, T, D), np.float32)
    for cix in range(N_CORES):
        b, qtok = metas[cix]
        tiles = results[cix]["out"]           # [128, FT, TQ]
        core_t = tiles.transpose(1, 0, 2).reshape(D, TQ)
        out[b, qtok, :] = core_t.T
    return out
